# revision 11
# baseline (speedup 1.0000x reference)
"""MetaPathAggregator kernel for Trainium2 (8 NeuronCores, data-parallel).

Math: the reference module is linear in the four gathered feature rows:

    dis  = 0.125*(mi+g1)@Wdd^T + 0.25*g2 + 0.5*dr
    drug = 0.125*(dr+g2)@Wdg^T + 0.25*g1 + 0.5*mi
    out  = [drug @ Wdrug^T | dis @ Wdis^T]
         = mi@M_mi + g1@M_g1 + g2@M_g2 + dr@M_dr

with per-slot 128x128 matrices

    M_mi = [0.500*C | 0.125*A]      A = Wdd^T @ Wdis^T   (128x64)
    M_g1 = [0.250*C | 0.125*A]      B = Wdg^T @ Wdrug^T  (128x64)
    M_g2 = [0.125*B | 0.250*D]      C = Wdrug^T          (128x64)
    M_dr = [0.125*B | 0.500*D]      D = Wdis^T           (128x64)

Since mp_ins indices are < 1000 (spec fill_max), only the first 1024 rows of
each feature table are live.  The kernel transforms the tables once on-device
(T_x = feat_x @ M_x, PE matmuls) and the per-token work collapses to four
row-gathers and three adds: out[t] = T_mi[i0]+T_g1[i1]+T_g2[i2]+T_dr[i3].

Device schedule per core (16384 tokens): prep (weights -> M matrices -> T
tables in DRAM scratch), then 8 chunks x (4 dma_gather of 2048 rows + 3 DVE
adds + 1 streaming store).  HBM traffic/core ~42MB => memory-bound.
"""

import numpy as np

P = 128          # partitions
F = 128          # input feature dim
H = 128          # output hidden dim
HH = 64          # half hidden
R = 1024         # padded table rows (indices < 1000)
N_CORES = 8
B_PAIRS = 1024
BAG = 128
TOK = B_PAIRS * BAG // N_CORES   # 16384 tokens per core
CH = 1024                        # tokens per chunk (1024 descs per dma_gather)
NCH = TOK // CH                  # 16 chunks
CPB = CH // P                    # 8 tokens per partition per chunk

_CACHE = {}


def _build_module(do_prep=True, do_gathers=True, do_adds=True, do_stores=True):
    import concourse.bacc as bacc
    import concourse.mybir as mybir
    import concourse.tile as tile
    from concourse.masks import make_identity
    from concourse.tile_rust import add_dep_helper

    f32 = mybir.dt.float32
    i16 = mybir.dt.int16

    nc = bacc.Bacc("TRN2", dynamic_dma_scratch_size=65536)

    feat_in = {
        "mi": nc.dram_tensor("feat_mi", [R, F], f32, kind="ExternalInput"),
        "ge": nc.dram_tensor("feat_ge", [R, F], f32, kind="ExternalInput"),
        "dr": nc.dram_tensor("feat_dr", [R, F], f32, kind="ExternalInput"),
    }
    w_dd = nc.dram_tensor("w_dd", [H, F], f32, kind="ExternalInput")
    w_dg = nc.dram_tensor("w_dg", [H, F], f32, kind="ExternalInput")
    w_drug = nc.dram_tensor("w_drug", [HH, F], f32, kind="ExternalInput")
    w_dis = nc.dram_tensor("w_dis", [HH, F], f32, kind="ExternalInput")
    idx_in = nc.dram_tensor("idx", [P, 4, NCH, CH // 16], i16, kind="ExternalInput")
    out = nc.dram_tensor("out", [TOK, H], f32, kind="ExternalOutput")

    with tile.TileContext(nc) as tc:
        with (
            tc.tile_pool(name="const", bufs=1) as cpool,
            tc.tile_pool(name="prep", bufs=2) as ppool,
            tc.tile_pool(name="psum", bufs=2, space="PSUM") as pspool,
            tc.tile_pool(name="tdram", bufs=1, space="DRAM") as dpool,
            tc.tile_pool(name="gather", bufs=4) as gpool,
        ):
            ident = cpool.tile([P, P], f32)
            make_identity(nc, ident[:])

            idx_t = cpool.tile([P, 4, NCH, CH // 16], i16)
            nc.sync.dma_start(idx_t[:], idx_in[:, :, :, :])

            # ---- load weights
            wdd_t = cpool.tile([H, F], f32, tag="wdd")
            nc.sync.dma_start(wdd_t[:], w_dd[:, :])
            wdg_t = cpool.tile([H, F], f32, tag="wdg")
            nc.sync.dma_start(wdg_t[:], w_dg[:, :])
            wdrug_t = cpool.tile([HH, F], f32, tag="wdrug")
            nc.sync.dma_start(wdrug_t[:], w_drug[:, :])
            wdis_t = cpool.tile([HH, F], f32, tag="wdis")
            nc.sync.dma_start(wdis_t[:], w_dis[:, :])

            # ---- C = Wdrug^T, D = Wdis^T  (PE transpose via identity)
            c_ps = pspool.tile([F, HH], f32, tag="tps")
            nc.tensor.transpose(out=c_ps[:], in_=wdrug_t[:], identity=ident[:HH, :HH])
            c_s = cpool.tile([F, HH], f32, tag="c_s")
            nc.vector.tensor_copy(out=c_s[:], in_=c_ps[:])

            d_ps = pspool.tile([F, HH], f32, tag="tps")
            nc.tensor.transpose(out=d_ps[:], in_=wdis_t[:], identity=ident[:HH, :HH])
            d_s = cpool.tile([F, HH], f32, tag="d_s")
            nc.vector.tensor_copy(out=d_s[:], in_=d_ps[:])

            # ---- A = Wdd^T @ Wdis^T, B = Wdg^T @ Wdrug^T
            a_ps = pspool.tile([F, HH], f32, tag="abps")
            nc.tensor.matmul(out=a_ps[:], lhsT=wdd_t[:], rhs=d_s[:], start=True, stop=True)
            b_ps = pspool.tile([F, HH], f32, tag="abps")
            nc.tensor.matmul(out=b_ps[:], lhsT=wdg_t[:], rhs=c_s[:], start=True, stop=True)

            # ---- assemble M matrices [F, H] in SBUF
            m = {k: cpool.tile([F, H], f32, tag=f"m_{k}", name=f"m_{k}") for k in range(4)}
            # slot 0 = mi, 1 = g1, 2 = g2, 3 = dr
            nc.vector.tensor_scalar_mul(m[0][:, :HH], c_s[:], 0.5)
            nc.vector.tensor_scalar_mul(m[0][:, HH:], a_ps[:], 0.125)
            nc.vector.tensor_scalar_mul(m[1][:, :HH], c_s[:], 0.25)
            nc.vector.tensor_scalar_mul(m[1][:, HH:], a_ps[:], 0.125)
            nc.vector.tensor_scalar_mul(m[2][:, :HH], b_ps[:], 0.125)
            nc.vector.tensor_scalar_mul(m[2][:, HH:], d_s[:], 0.25)
            nc.vector.tensor_scalar_mul(m[3][:, :HH], b_ps[:], 0.125)
            nc.vector.tensor_scalar_mul(m[3][:, HH:], d_s[:], 0.5)

            # ---- transform tables: T_k = feat @ M_k  -> DRAM scratch
            NT = R // P  # 8 row-tiles per table
            t_dram = [dpool.tile([R, F], f32, tag=f"t{k}", name=f"t_dram{k}") for k in range(4)]
            t_store = [None] * 4  # store instruction per table (for gather deps)

            # which feature table feeds each slot
            slot_feat = {0: "mi", 1: "ge", 2: "ge", 3: "dr"}

            feat_tiles = {}
            for name, hbm in feat_in.items():
                ft = cpool.tile([P, NT, F], f32, tag=f"feat_{name}", name=f"feat_tile_{name}")
                nc.sync.dma_start(
                    ft[:], hbm[:, :].rearrange("(r p) f -> p r f", p=P)
                )
                feat_tiles[name] = ft

            # per feature table: transpose row-tiles, then transform every slot
            # that uses it (gene feeds both g1 and g2) and store to DRAM.
            # Ordered per table so early tables' gathers can start during prep.
            feat_slots = {"mi": [0], "ge": [1, 2], "dr": [3]}
            staged = {k: ppool.tile([P, NT, F], f32, tag=f"tstage{k}", name=f"tstage{k}")
                      for k in range(4)}
            for name in ("mi", "dr", "ge"):
                for r in range(NT):
                    tp = pspool.tile([P, P], f32, tag="ftps")
                    nc.tensor.transpose(
                        out=tp[:], in_=feat_tiles[name][:, r, :], identity=ident[:]
                    )
                    fts = ppool.tile([P, P], f32, tag="ftT", name=f"ftT_{name}_{r}",
                                     bufs=3)
                    # alternate PSUM->SBUF copies between DVE and ACT
                    if r % 2 == 0:
                        nc.vector.tensor_copy(out=fts[:], in_=tp[:])
                    else:
                        nc.scalar.activation(
                            out=fts[:], in_=tp[:],
                            func=mybir.ActivationFunctionType.Copy,
                        )
                    for k in feat_slots[name]:
                        mm = pspool.tile([P, H], f32, tag="mmps")
                        nc.tensor.matmul(
                            out=mm[:], lhsT=fts[:], rhs=m[k][:],
                            start=True, stop=True,
                        )
                        if k % 2 == 0:
                            nc.vector.tensor_copy(out=staged[k][:, r, :], in_=mm[:])
                        else:
                            nc.scalar.activation(
                                out=staged[k][:, r, :], in_=mm[:],
                                func=mybir.ActivationFunctionType.Copy,
                            )
                for k in feat_slots[name]:
                    t_store[k] = nc.sync.dma_start(
                        t_dram[k][:, :].rearrange("(r p) f -> p r f", p=P),
                        staged[k][:],
                    )

            # ---- main loop: gather + add + store
            # The Pool engine runs gathers in emission order.  The gene table
            # (slots 1,2) finishes its transform last, so its gathers are
            # delayed by GE_DELAY chunks relative to mi/dr gathers -- the
            # Pool engine streams ready mi/dr gathers instead of stalling at
            # the head of the queue waiting for the gene T table.
            GE_DELAY = 5
            gtiles = {}

            def issue_gather(k, ch):
                bufs = GE_DELAY + 2 if k in (0, 3) else 3
                gt = gpool.tile([P, CPB, F], f32, tag=f"g{k}", name=f"g{k}_{ch}",
                                bufs=bufs)
                if do_gathers:
                    gi = nc.gpsimd.dma_gather(
                        gt[:], t_dram[k][:, :], idx_t[:, k, ch, :], CH, CH, F,
                    )
                    add_dep_helper(gi.ins, t_store[k].ins, reason="gather after T store")
                gtiles[(k, ch)] = gt

            for ch in range(NCH + GE_DELAY):
                if ch < NCH:
                    issue_gather(0, ch)
                    issue_gather(3, ch)
                ch2 = ch - GE_DELAY
                if 0 <= ch2 < NCH:
                    issue_gather(1, ch2)
                    issue_gather(2, ch2)
                    g = [gtiles[(k, ch2)] for k in range(4)]
                    if do_adds:
                        nc.vector.tensor_add(g[0][:], g[0][:], g[1][:])
                        nc.vector.tensor_add(g[2][:], g[2][:], g[3][:])
                        nc.vector.tensor_add(g[0][:], g[0][:], g[2][:])
                    if do_stores:
                        nc.sync.dma_start(
                            out[ch2 * CH : (ch2 + 1) * CH, :].rearrange(
                                "(p s) h -> p s h", p=P),
                            g[0][:],
                        )

    nc.compile()
    return nc


def _prep_inputs(feat_miRNA, feat_gene, feat_drug, W_drug_disease, W_disease_drug,
                 W_drug, W_dis, mp_ins):
    """Marshal full inputs into per-core in_maps (no arithmetic on values)."""
    def pad_rows(a):
        a = np.ascontiguousarray(np.asarray(a, dtype=np.float32))
        if a.shape[0] >= R:
            return np.ascontiguousarray(a[:R])
        out = np.zeros((R, a.shape[1]), dtype=np.float32)
        out[: a.shape[0]] = a
        return out

    f_mi = pad_rows(feat_miRNA)
    f_ge = pad_rows(feat_gene)
    f_dr = pad_rows(feat_drug)
    wdd = np.ascontiguousarray(np.asarray(W_drug_disease, np.float32))
    wdg = np.ascontiguousarray(np.asarray(W_disease_drug, np.float32))
    wdrug = np.ascontiguousarray(np.asarray(W_drug, np.float32))
    wdis = np.ascontiguousarray(np.asarray(W_dis, np.float32))

    mp = np.asarray(mp_ins)
    assert mp.shape == (B_PAIRS, BAG, 4), mp.shape

    # gather-slot permutation: out[p, s] holds token p*CPB+s of the chunk;
    # gather slot j = s*128+p; wrapped idx layout: j -> [j%16, j//16], x8 groups
    j = np.arange(CH)
    tok_of_j = (j % P) * CPB + (j // P)          # token within chunk for slot j

    in_maps = []
    for core in range(N_CORES):
        mp_core = mp[core * (B_PAIRS // N_CORES) : (core + 1) * (B_PAIRS // N_CORES)]
        mp_core = mp_core.reshape(TOK, 4).astype(np.int16)
        idx_arr = np.empty((P, 4, NCH, CH // 16), dtype=np.int16)
        for ch in range(NCH):
            t = ch * CH + tok_of_j                 # absolute token per slot j
            for k in range(4):
                lin = mp_core[t, k]                # idx for gather slot j
                wrapped = lin.reshape(CH // 16, 16).T   # [16, CH/16]
                idx_arr[:, k, ch, :] = np.tile(wrapped, (8, 1))
        in_maps.append(
            {
                "feat_mi": f_mi,
                "feat_ge": f_ge,
                "feat_dr": f_dr,
                "w_dd": wdd,
                "w_dg": wdg,
                "w_drug": wdrug,
                "w_dis": wdis,
                "idx": idx_arr,
            }
        )
    return in_maps


def _numpy_fallback(feat_miRNA, feat_gene, feat_drug, W_drug_disease,
                    W_disease_drug, W_drug, W_dis, mp_ins):
    mi = np.asarray(feat_miRNA, np.float32)[mp_ins[:, :, 0]]
    g1 = np.asarray(feat_gene, np.float32)[mp_ins[:, :, 1]]
    g2 = np.asarray(feat_gene, np.float32)[mp_ins[:, :, 2]]
    dr = np.asarray(feat_drug, np.float32)[mp_ins[:, :, 3]]
    wdd = np.asarray(W_drug_disease, np.float32)
    wdg = np.asarray(W_disease_drug, np.float32)
    wdrug = np.asarray(W_drug, np.float32)
    wdis = np.asarray(W_dis, np.float32)
    dis = ((((mi + g1) * 0.5) @ wdd.T + g2) * 0.5 + dr) * 0.5
    drug = ((((dr + g2) * 0.5) @ wdg.T + g1) * 0.5 + mi) * 0.5
    return np.concatenate([drug @ wdrug.T, dis @ wdis.T], axis=2)


def kernel(**inputs):
    mp = np.asarray(inputs["mp_ins"])
    if mp.max() >= R or mp.min() < 0:
        # outside the spec's index range; fall back to exact host compute
        return _numpy_fallback(**inputs)

    from concourse.bass_utils import run_bass_kernel_spmd

    if "nc" not in _CACHE:
        _CACHE["nc"] = _build_module()
    nc = _CACHE["nc"]

    in_maps = _prep_inputs(**inputs)
    res = run_bass_kernel_spmd(nc, in_maps, core_ids=list(range(N_CORES)))
    outs = [r["out"] for r in res.results]
    return np.concatenate(outs, axis=0).reshape(B_PAIRS, BAG, H)


if __name__ == "__main__":
    import reference

    inputs = {k: np.asarray(v) for k, v in reference.setup_inputs().items()}
    expected = np.asarray(reference.reference(**inputs))
    actual = kernel(**inputs)
    err = np.abs(actual - expected).max() / (np.abs(expected).max() + 1e-9)
    print("max abs err (scaled):", err)
    rel = np.linalg.norm(actual - expected) / np.linalg.norm(expected)
    print("Relative error:", rel)
